# revision 13
# baseline (speedup 1.0000x reference)
"""Single-head causal attention (B=4, S=4096, E=1024, H=128) on 8 TRN2 cores.

Sharding: 2 cores per batch. Each pair splits every query-tile's causal
key-range by interleaved 256-wide key chunks (even chunks -> core 2b, odd
chunks -> core 2b+1). Both cores of a pair run the IDENTICAL program over
all 8 query tiles of their batch; only the input data (a host-permuted
x^T) differs. Unnormalized flash partials (O^T, l) are combined on the
host: O = (O_A + O_B) / (l_A + l_B).

Device pipeline per core:
  - Q^T/K^T/V^T projections from x^T (E on partitions) via fp32r matmuls
  - V^T -> V via exact fp32 PE transposes
  - per q-tile: S^T = K_blk^T^T.T @ Q^T (PSUM), exp via ACT (scale=H^-0.5)
    -> P^T (SBUF, fp32r), causal diag masking via DVE mul with host masks,
    O^T += V_blk.T @ P^T and l += ones.T @ P^T accumulated in PSUM,
    then DMA PSUM -> DRAM.
"""

import numpy as np

BS, SL, ES, HS = 4, 4096, 1024, 128
NCORES = 8
CH = 256             # key interleave chunk width
NQT = SL // 512      # 8 query tiles of 512
KHALF = SL // 2      # 2048 per-core key columns
SCALE = float(HS) ** -0.5

_CACHE = {}


def _build_module():
    import concourse.bass as bass  # noqa: F401
    import concourse.mybir as mybir
    import concourse.tile as tile
    from concourse import bacc
    from concourse.masks import make_identity

    dt = mybir.dt
    nc = bacc.Bacc("TRN2", target_bir_lowering=False, debug=False,
                   num_devices=NCORES)

    xT = nc.dram_tensor("xT", [ES, SL], dt.float32r, kind="ExternalInput")
    wq = nc.dram_tensor("wq", [ES, HS], dt.float32r, kind="ExternalInput")
    wk = nc.dram_tensor("wk", [ES, HS], dt.float32r, kind="ExternalInput")
    wv = nc.dram_tensor("wv", [ES, HS], dt.float32r, kind="ExternalInput")
    msk = nc.dram_tensor("msk", [128, 1024], dt.float32r, kind="ExternalInput")
    oT = nc.dram_tensor("oT", [HS, SL], dt.float32, kind="ExternalOutput")
    lout = nc.dram_tensor("lout", [1, SL], dt.float32, kind="ExternalOutput")

    ECH = ES // 128  # 8 e-chunks

    with tile.TileContext(nc) as tc:
        with (
            tc.tile_pool(name="const", bufs=1) as constp,
            tc.tile_pool(name="wpool", bufs=1) as wpool,
            tc.tile_pool(name="xstage", bufs=3) as xstage,
            tc.tile_pool(name="proj", bufs=1) as projp,
            tc.tile_pool(name="ppool", bufs=8) as ppool,
            tc.tile_pool(name="opool", bufs=2) as opool,
            tc.tile_pool(name="pspr", bufs=2, space="PSUM") as pspr,
            tc.tile_pool(name="psS", bufs=2, space="PSUM") as psS,
            tc.tile_pool(name="pso", bufs=1, space="PSUM") as pso,
            tc.tile_pool(name="psl", bufs=1, space="PSUM") as psl,
        ):
            ident = constp.tile([128, 128], dt.float32)
            make_identity(nc, ident[:])
            ones32 = constp.tile([128, 1], dt.float32)
            nc.any.memset(ones32[:], 1.0)
            ones = constp.tile([128, 1], dt.float32r)
            nc.vector.tensor_copy(ones[:], ones32[:])
            xT_r = xT.ap().rearrange("(c p) s -> p c s", p=128)  # [128, 8, SL]
            w_sb = {}
            for name, w in (("q", wq), ("k", wk), ("v", wv)):
                t = wpool.tile([128, ECH, HS], dt.float32r, tag=f"w{name}")
                nc.sync.dma_start(t[:], w.ap().rearrange("(c p) h -> p c h", p=128))
                w_sb[name] = t
                if name == "q":
                    xt0 = xstage.tile([128, ECH, 512], dt.float32r, tag="xt",
                                      name="xt0")
                    for c in range(ECH):
                        nc.sync.dma_start(xt0[:, c, :], xT_r[:, c, 0:512])
            masks = constp.tile([128, 1024], dt.float32r)
            nc.sync.dma_start(masks[:], msk[:])

            qT = [projp.tile([128, 512], dt.float32r, tag=f"qT{i}", name=f"qT{i}")
                  for i in range(NQT)]
            kT = [projp.tile([128, 256], dt.float32r, tag=f"kT{i}", name=f"kT{i}")
                  for i in range(NQT)]
            vT = [projp.tile([128, 256], dt.float32, tag=f"vT{i}", name=f"vT{i}")
                  for i in range(NQT)]
            vn = [projp.tile([128, 128], dt.float32r, tag=f"vn{i}", name=f"vn{i}")
                  for i in range(16)]

            # ---- attention tile emitter ----
            def attention_steps(t):
                o_acc = pso.tile([128, 512], dt.float32, tag="oacc",
                                 name=f"oacc{t}")
                l_acc = psl.tile([1, 512], dt.float32, tag="lacc",
                                 name=f"lacc{t}")
                nkb = 2 * (t + 1)
                p_tiles = []

                def chunk_step(c):
                    s2 = psS.tile([128, 1024], dt.float32, tag="s2",
                                  name=f"s2_{t}_{c}")
                    for d in range(2):
                        kb = 2 * c + d
                        lhs = kT[c][:, d * 128:(d + 1) * 128]
                        nc.tensor.matmul(s2[:, d * 512:(d + 1) * 512], lhs,
                                         qT[t][:], start=True, stop=True)
                    p2 = ppool.tile([128, 1024], dt.float32r, tag="p",
                                    name=f"p{t}_{c}")
                    nc.scalar.activation(p2[:], s2[:],
                                         mybir.ActivationFunctionType.Exp,
                                         scale=SCALE)
                    if c == t:  # diagonal chunk: causal mask (both halves)
                        nc.vector.tensor_mul(p2[:], p2[:], masks[:])
                    for d in range(2):
                        kb = 2 * c + d
                        nc.tensor.matmul(o_acc[:], vn[kb][:],
                                         p2[:, d * 512:(d + 1) * 512],
                                         start=(kb == 0), stop=(kb == nkb - 1))
                    p_tiles.append(p2)

                def finish():
                    for kb in range(nkb):
                        nc.tensor.matmul(l_acc[:], ones[:],
                                         p_tiles[kb // 2][:, (kb % 2) * 512:
                                                          (kb % 2 + 1) * 512],
                                         start=(kb == 0), stop=(kb == nkb - 1))
                    o_sb = opool.tile([128, 512], dt.float32, tag="osb",
                                      name=f"osb{t}")
                    nc.vector.tensor_copy(o_sb[:], o_acc[:])
                    l_sb = opool.tile([1, 512], dt.float32, tag="lsb",
                                      name=f"lsb{t}")
                    nc.vector.tensor_copy(l_sb[:], l_acc[:])
                    nc.sync.dma_start(oT.ap()[:, t * 512:(t + 1) * 512], o_sb[:])
                    nc.sync.dma_start(lout.ap()[:, t * 512:(t + 1) * 512],
                                      l_sb[:])

                return [lambda c=c: chunk_step(c) for c in range(t + 1)] + [finish]

            # ---- projections interleaved with attention ----
            for st in range(NQT):
                steps = attention_steps(st - 1) if st >= 1 else []
                cut = [(len(steps) * i) // 4 for i in range(5)]

                def emit(i):
                    for f in steps[cut[i]:cut[i + 1]]:
                        f()

                if st == 0:
                    xt = xt0
                else:
                    xt = xstage.tile([128, ECH, 512], dt.float32r, tag="xt")
                    for c in range(ECH):
                        nc.sync.dma_start(xt[:, c, :],
                                          xT_r[:, c, st * 512:(st + 1) * 512])

                qp = pspr.tile([128, 512], dt.float32, tag="pr")
                for c in range(ECH):
                    nc.tensor.matmul(qp[:], w_sb["q"][:, c, :], xt[:, c, :],
                                     start=(c == 0), stop=(c == ECH - 1))
                nc.vector.tensor_copy(qT[st][:], qp[:])
                emit(0)

                kp = pspr.tile([128, 512], dt.float32, tag="pr")
                for c in range(ECH):
                    nc.tensor.matmul(kp[:, 0:256], w_sb["k"][:, c, :],
                                     xt[:, c, 0:256],
                                     start=(c == 0), stop=(c == ECH - 1))
                nc.vector.tensor_copy(kT[st][:], kp[:, 0:256])
                emit(1)

                vp = pspr.tile([128, 512], dt.float32, tag="pr")
                for c in range(ECH):
                    nc.tensor.matmul(vp[:, 0:256], w_sb["v"][:, c, :],
                                     xt[:, c, 0:256],
                                     start=(c == 0), stop=(c == ECH - 1))
                nc.vector.tensor_copy(vT[st][:], vp[:, 0:256])
                emit(2)

                # transpose V^T -> V natural for the 2 k128-blocks
                for jj in range(2):
                    j = st * 2 + jj
                    tp = pspr.tile([128, 512], dt.float32, tag="pr")
                    nc.tensor.transpose(tp[:, 0:128],
                                        vT[st][:, jj * 128:(jj + 1) * 128],
                                        ident[:])
                    nc.vector.tensor_copy(vn[j][:], tp[:, 0:128])

                emit(3)

            for f in attention_steps(NQT - 1):
                f()

    nc.finalize()
    return nc


def _host_inputs(x, Wq, Wk, Wv):
    """Per-core input maps. Core 2b+d gets batch b, key-parity d."""
    nhalf = SL // CH // 2  # 8 chunks per half
    # masks [128, 1024]: two 512-wide masks for diag k128-blocks d=0,1
    p = np.arange(128, dtype=np.int64)[:, None]
    f = np.arange(512, dtype=np.int64)[None, :]
    mA = np.concatenate([(f >= p), (f >= 128 + p)], axis=1).astype(np.float32)
    mB = np.concatenate([(f >= p) & (f < 256), (f >= 128 + p) & (f < 256)],
                        axis=1).astype(np.float32)
    in_maps = []
    for core in range(NCORES):
        b, d = core // 2, core % 2
        xTb = np.ascontiguousarray(x[b].T)           # [ES, SL]
        if d == 0:
            xTperm = xTb
        else:
            chunks = xTb.reshape(ES, SL // CH // 2, 2, CH)
            xTperm = np.ascontiguousarray(
                chunks[:, :, ::-1, :].reshape(ES, SL))
        in_maps.append({
            "xT": xTperm,
            "wq": np.ascontiguousarray(Wq),
            "wk": np.ascontiguousarray(Wk),
            "wv": np.ascontiguousarray(Wv),
            "msk": mA if d == 0 else mB,
        })
    return in_maps


def _host_combine(results):
    """Combine per-core (O^T, l) partials into [BS, SL, HS] output."""
    out = np.empty((BS, SL, HS), dtype=np.float32)
    for b in range(BS):
        oA = results[2 * b]["oT"]
        lA = results[2 * b]["lout"]
        oB = results[2 * b + 1]["oT"]
        lB = results[2 * b + 1]["lout"]
        # core B's q columns are half-swapped within each 512 tile
        oBn = oB.reshape(HS, NQT, 2, 256)[:, :, ::-1, :].reshape(HS, SL)
        lBn = lB.reshape(1, NQT, 2, 256)[:, :, ::-1, :].reshape(1, SL)
        o = oA + oBn
        l = lA + lBn
        out[b] = (o / l).T
    return out


def kernel(x, Wq, Wk, Wv, _trace=False):
    from concourse.bass_utils import run_bass_kernel_spmd

    if "nc" not in _CACHE:
        _CACHE["nc"] = _build_module()
    nc = _CACHE["nc"]
    in_maps = _host_inputs(np.asarray(x, dtype=np.float32),
                           np.asarray(Wq, dtype=np.float32),
                           np.asarray(Wk, dtype=np.float32),
                           np.asarray(Wv, dtype=np.float32))
    res = run_bass_kernel_spmd(nc, in_maps, core_ids=list(range(NCORES)),
                               trace=_trace)
    out = _host_combine(res.results)
    if _trace:
        _CACHE["last_result"] = res
    return out


# revision 14
# speedup vs baseline: 1.0798x; 1.0798x over previous
"""Single-head causal attention (B=4, S=4096, E=1024, H=128) on 8 TRN2 cores.

Sharding: 2 cores per batch. Each pair splits every query-tile's causal
key-range by interleaved 256-wide key chunks (even chunks -> core 2b, odd
chunks -> core 2b+1). Both cores of a pair run the IDENTICAL program over
all 8 query tiles of their batch; only the input data (a host-permuted
x^T) differs. Unnormalized flash partials (O^T, l) are combined on the
host: O = (O_A + O_B) / (l_A + l_B).

Device pipeline per core:
  - Q^T/K^T/V^T projections from x^T (E on partitions) via fp32r matmuls
  - V^T -> V via exact fp32 PE transposes
  - per q-tile: S^T = K_blk^T^T.T @ Q^T (PSUM), exp via ACT (scale=H^-0.5)
    -> P^T (SBUF, fp32r), causal diag masking via DVE mul with host masks,
    O^T += V_blk.T @ P^T and l += ones.T @ P^T accumulated in PSUM,
    then DMA PSUM -> DRAM.
"""

import numpy as np

BS, SL, ES, HS = 4, 4096, 1024, 128
NCORES = 8
CH = 256             # key interleave chunk width
NQT = SL // 512      # 8 query tiles of 512
KHALF = SL // 2      # 2048 per-core key columns
SCALE = float(HS) ** -0.5

_CACHE = {}


def _build_module():
    import concourse.bass as bass  # noqa: F401
    import concourse.mybir as mybir
    import concourse.tile as tile
    from concourse import bacc
    from concourse.masks import make_identity

    dt = mybir.dt
    nc = bacc.Bacc("TRN2", target_bir_lowering=False, debug=False,
                   num_devices=NCORES)

    xT = nc.dram_tensor("xT", [ES, SL], dt.float32r, kind="ExternalInput")
    wq = nc.dram_tensor("wq", [ES, HS], dt.float32r, kind="ExternalInput")
    wk = nc.dram_tensor("wk", [ES, HS], dt.float32r, kind="ExternalInput")
    wv = nc.dram_tensor("wv", [ES, HS], dt.float32r, kind="ExternalInput")
    msk = nc.dram_tensor("msk", [128, 1024], dt.float32r, kind="ExternalInput")
    oT = nc.dram_tensor("oT", [HS, SL], dt.float32, kind="ExternalOutput")
    lout = nc.dram_tensor("lout", [1, SL], dt.float32, kind="ExternalOutput")

    ECH = ES // 128  # 8 e-chunks

    with tile.TileContext(nc) as tc:
        with (
            tc.tile_pool(name="const", bufs=1) as constp,
            tc.tile_pool(name="wpool", bufs=1) as wpool,
            tc.tile_pool(name="xstage", bufs=3) as xstage,
            tc.tile_pool(name="proj", bufs=1) as projp,
            tc.tile_pool(name="ppool", bufs=16) as ppool,
            tc.tile_pool(name="opool", bufs=2) as opool,
            tc.tile_pool(name="pspr", bufs=5, space="PSUM") as pspr,
            tc.tile_pool(name="pso", bufs=2, space="PSUM") as pso,
            tc.tile_pool(name="psl", bufs=1, space="PSUM") as psl,
        ):
            ident = constp.tile([128, 128], dt.float32)
            make_identity(nc, ident[:])
            ones32 = constp.tile([128, 1], dt.float32)
            nc.any.memset(ones32[:], 1.0)
            ones = constp.tile([128, 1], dt.float32r)
            nc.vector.tensor_copy(ones[:], ones32[:])
            xT_r = xT.ap().rearrange("(c p) s -> p c s", p=128)  # [128, 8, SL]
            w_sb = {}
            for name, w in (("q", wq), ("k", wk), ("v", wv)):
                t = wpool.tile([128, ECH, HS], dt.float32r, tag=f"w{name}")
                nc.sync.dma_start(t[:], w.ap().rearrange("(c p) h -> p c h", p=128))
                w_sb[name] = t
                if name == "q":
                    xt0 = xstage.tile([128, ECH, 512], dt.float32r, tag="xt",
                                      name="xt0")
                    for c in range(ECH):
                        nc.sync.dma_start(xt0[:, c, :], xT_r[:, c, 0:512])
            masks = constp.tile([128, 1024], dt.float32r)
            nc.sync.dma_start(masks[:], msk[:])

            qT = [projp.tile([128, 512], dt.float32r, tag=f"qT{i}", name=f"qT{i}")
                  for i in range(NQT)]
            kT = [projp.tile([128, 256], dt.float32r, tag=f"kT{i}", name=f"kT{i}")
                  for i in range(NQT)]
            vT = [projp.tile([128, 256], dt.float32, tag=f"vT{i}", name=f"vT{i}")
                  for i in range(NQT)]
            vn = [projp.tile([128, 128], dt.float32r, tag=f"vn{i}", name=f"vn{i}")
                  for i in range(16)]

            # ---- attention tile emitter ----
            def attention_steps(t):
                o_acc = pso.tile([128, 512], dt.float32, tag="oacc",
                                 name=f"oacc{t}")
                l_acc = psl.tile([1, 512], dt.float32, tag="lacc",
                                 name=f"lacc{t}")
                nkb = 2 * (t + 1)
                p_tiles = []

                def kb_step(kb):
                    c, d = kb // 2, kb % 2
                    s_ps = pspr.tile([128, 512], dt.float32, tag="pr",
                                     name=f"sps{t}_{kb}")
                    lhs = kT[kb // 2][:, (kb % 2) * 128:(kb % 2 + 1) * 128]
                    nc.tensor.matmul(s_ps[:], lhs, qT[t][:],
                                     start=True, stop=True)
                    p_sb = ppool.tile([128, 512], dt.float32r, tag="p",
                                      name=f"p{t}_{kb}")
                    nc.scalar.activation(p_sb[:], s_ps[:],
                                         mybir.ActivationFunctionType.Exp,
                                         scale=SCALE)
                    if c == t:  # diagonal chunk: causal mask
                        nc.vector.tensor_mul(
                            p_sb[:], p_sb[:],
                            masks[:, d * 512:(d + 1) * 512])
                    nc.tensor.matmul(o_acc[:], vn[kb][:], p_sb[:],
                                     start=(kb == 0), stop=(kb == nkb - 1))
                    p_tiles.append(p_sb)

                def finish():
                    for kb in range(nkb):
                        nc.tensor.matmul(l_acc[:], ones[:], p_tiles[kb][:],
                                         start=(kb == 0), stop=(kb == nkb - 1))
                    o_sb = opool.tile([128, 512], dt.float32, tag="osb",
                                      name=f"osb{t}")
                    nc.vector.tensor_copy(o_sb[:], o_acc[:])
                    l_sb = opool.tile([1, 512], dt.float32, tag="lsb",
                                      name=f"lsb{t}")
                    nc.vector.tensor_copy(l_sb[:], l_acc[:])
                    nc.sync.dma_start(oT.ap()[:, t * 512:(t + 1) * 512], o_sb[:])
                    nc.sync.dma_start(lout.ap()[:, t * 512:(t + 1) * 512],
                                      l_sb[:])

                return [lambda kb=kb: kb_step(kb) for kb in range(nkb)] + [finish]

            # ---- projections interleaved with attention ----
            for st in range(NQT):
                steps = attention_steps(st - 1) if st >= 1 else []
                cut = [(len(steps) * i) // 4 for i in range(5)]

                def emit(i):
                    for f in steps[cut[i]:cut[i + 1]]:
                        f()

                if st == 0:
                    xt = xt0
                else:
                    xt = xstage.tile([128, ECH, 512], dt.float32r, tag="xt")
                    for c in range(ECH):
                        nc.sync.dma_start(xt[:, c, :],
                                          xT_r[:, c, st * 512:(st + 1) * 512])

                qp = pspr.tile([128, 512], dt.float32, tag="pr")
                for c in range(ECH):
                    nc.tensor.matmul(qp[:], w_sb["q"][:, c, :], xt[:, c, :],
                                     start=(c == 0), stop=(c == ECH - 1))
                nc.vector.tensor_copy(qT[st][:], qp[:])
                emit(0)

                kp = pspr.tile([128, 512], dt.float32, tag="pr")
                for c in range(ECH):
                    nc.tensor.matmul(kp[:, 0:256], w_sb["k"][:, c, :],
                                     xt[:, c, 0:256],
                                     start=(c == 0), stop=(c == ECH - 1))
                nc.vector.tensor_copy(kT[st][:], kp[:, 0:256])
                emit(1)

                vp = pspr.tile([128, 512], dt.float32, tag="pr")
                for c in range(ECH):
                    nc.tensor.matmul(vp[:, 0:256], w_sb["v"][:, c, :],
                                     xt[:, c, 0:256],
                                     start=(c == 0), stop=(c == ECH - 1))
                nc.vector.tensor_copy(vT[st][:], vp[:, 0:256])
                emit(2)

                # transpose V^T -> V natural for the 2 k128-blocks
                for jj in range(2):
                    j = st * 2 + jj
                    tp = pspr.tile([128, 512], dt.float32, tag="pr")
                    nc.tensor.transpose(tp[:, 0:128],
                                        vT[st][:, jj * 128:(jj + 1) * 128],
                                        ident[:])
                    nc.vector.tensor_copy(vn[j][:], tp[:, 0:128])

                emit(3)

            for f in attention_steps(NQT - 1):
                f()

    nc.finalize()
    return nc


def _host_inputs(x, Wq, Wk, Wv):
    """Per-core input maps. Core 2b+d gets batch b, key-parity d."""
    nhalf = SL // CH // 2  # 8 chunks per half
    # masks [128, 1024]: two 512-wide masks for diag k128-blocks d=0,1
    p = np.arange(128, dtype=np.int64)[:, None]
    f = np.arange(512, dtype=np.int64)[None, :]
    mA = np.concatenate([(f >= p), (f >= 128 + p)], axis=1).astype(np.float32)
    mB = np.concatenate([(f >= p) & (f < 256), (f >= 128 + p) & (f < 256)],
                        axis=1).astype(np.float32)
    in_maps = []
    for core in range(NCORES):
        b, d = core // 2, core % 2
        xTb = np.ascontiguousarray(x[b].T)           # [ES, SL]
        if d == 0:
            xTperm = xTb
        else:
            chunks = xTb.reshape(ES, SL // CH // 2, 2, CH)
            xTperm = np.ascontiguousarray(
                chunks[:, :, ::-1, :].reshape(ES, SL))
        in_maps.append({
            "xT": xTperm,
            "wq": np.ascontiguousarray(Wq),
            "wk": np.ascontiguousarray(Wk),
            "wv": np.ascontiguousarray(Wv),
            "msk": mA if d == 0 else mB,
        })
    return in_maps


def _host_combine(results):
    """Combine per-core (O^T, l) partials into [BS, SL, HS] output."""
    out = np.empty((BS, SL, HS), dtype=np.float32)
    for b in range(BS):
        oA = results[2 * b]["oT"]
        lA = results[2 * b]["lout"]
        oB = results[2 * b + 1]["oT"]
        lB = results[2 * b + 1]["lout"]
        # core B's q columns are half-swapped within each 512 tile
        oBn = oB.reshape(HS, NQT, 2, 256)[:, :, ::-1, :].reshape(HS, SL)
        lBn = lB.reshape(1, NQT, 2, 256)[:, :, ::-1, :].reshape(1, SL)
        o = oA + oBn
        l = lA + lBn
        out[b] = (o / l).T
    return out


def kernel(x, Wq, Wk, Wv, _trace=False):
    from concourse.bass_utils import run_bass_kernel_spmd

    if "nc" not in _CACHE:
        _CACHE["nc"] = _build_module()
    nc = _CACHE["nc"]
    in_maps = _host_inputs(np.asarray(x, dtype=np.float32),
                           np.asarray(Wq, dtype=np.float32),
                           np.asarray(Wk, dtype=np.float32),
                           np.asarray(Wv, dtype=np.float32))
    res = run_bass_kernel_spmd(nc, in_maps, core_ids=list(range(NCORES)),
                               trace=_trace)
    out = _host_combine(res.results)
    if _trace:
        _CACHE["last_result"] = res
    return out
